# revision 55
# baseline (speedup 1.0000x reference)
"""CapsuleLinear (k-means routing) Trainium2 kernel.

Math: priors[b,o,i,j] = sum_l w[o,j,l] x[b,i,l]; 3 rounds of k-means routing
over in_capsules, squash=False.

priors is never materialized.  With G_o = W_o^T W_o (64x64 per out-capsule,
computed on-device once):

    u_0[b,l]   = sum_i x[b,i,l]                  (scale of u is irrelevant)
    per iter:  p = G_o u;  q = u.p = ||W u||^2
               rq = rsqrt(q) via DVE bit-hack + Newton (ACT keeps one table)
               v = p * rq           (v = W^T out_normalized)
               logits[i,o] = sum_l x[b,i,l] v[o,l]
               e = exp(logits)      (softmax Z cancels in v)
               u[o,l] = sum_i e[i,o] x[b,i,l];  Z[o] = sum_i e[i,o]
    output:    out[b,o,:] = W_o u_3[o,:] / Z_3[o]

Sharding: data-parallel over batch, 4 samples/core x 8 cores, weight
replicated, no collectives.  Host passes pre-transposed bf16 layouts
(xb with a ones column, xT, w as (j,o,l) and (l,o,j)) so the device does
no dtype conversion or weight transposition.
"""

import sys

if "/opt/trn_rl_repo" not in sys.path:
    sys.path.insert(0, "/opt/trn_rl_repo")

from contextlib import ExitStack

import ml_dtypes
import numpy as np

import concourse.bacc as bacc
import concourse.bass as bass
import concourse.bass_utils as bass_utils
import concourse.mybir as mybir
import concourse.tile as tile
from concourse.masks import make_identity

BF = mybir.dt.bfloat16
F32 = mybir.dt.float32
AF = mybir.ActivationFunctionType
ALU = mybir.AluOpType

B_GLOBAL = 32
N_CORES = 8
B = B_GLOBAL // N_CORES  # 4 samples per core
O = 128   # out_capsules
I = 512   # in_capsules
J = 64    # out_length
L = 64    # in_length
C = 4     # i-chunks of 128
NITER = 3
WCH = 8   # w DMA chunks
OCH = O // WCH


def _body(ctx: ExitStack, tc: "tile.TileContext", xb_d, xT_d, wj_d, wt_d,
          out_d, probe=None):
    nc = tc.nc

    const_pool = ctx.enter_context(tc.tile_pool(name="const", bufs=1))
    big = ctx.enter_context(tc.tile_pool(name="big", bufs=1))
    sb = ctx.enter_context(tc.tile_pool(name="sb", bufs=2))
    sbE = ctx.enter_context(tc.tile_pool(name="sbE", bufs=3))
    psP = ctx.enter_context(tc.tile_pool(name="psP", bufs=2, space="PSUM"))
    psL = ctx.enter_context(tc.tile_pool(name="psL", bufs=2, space="PSUM"))
    psU = ctx.enter_context(tc.tile_pool(name="psU", bufs=2, space="PSUM"))
    psQ = ctx.enter_context(tc.tile_pool(name="psQ", bufs=1, space="PSUM"))
    psB = ctx.enter_context(tc.tile_pool(name="psB", bufs=1, space="PSUM"))

    # ---- input DMAs (w first: the G pipeline is the head's long pole;
    # x lands while G is still streaming, wt is final-phase only)
    wj_tiles = []
    xb_sb = big.tile([128, B, C, L + 1], BF)
    for k in range(WCH):
        wj_k = big.tile([J, OCH, L], BF, tag=f"wj_{k}")
        nc.sync.dma_start(wj_k[:], wj_d[:, bass.ts(k, OCH), :])
        wj_tiles.append(wj_k)
        if k == WCH // 2 - 1:
            # xb mid-stream so u0 (hence iteration-1's half-0 q chain)
            # unblocks while the second-half w chunks still stream
            nc.sync.dma_start(xb_sb[:], xb_d)
    xT_sb = big.tile([L, B, C, 128], BF)
    nc.sync.dma_start(xT_sb[:], xT_d)
    wt_sb = big.tile([L, O, J], BF)
    nc.sync.dma_start(wt_sb[:], wt_d)

    def wj_ap(o):
        return wj_tiles[o // OCH][:, o % OCH, :]

    # ---- constants ----
    ident_bf = const_pool.tile([128, 128], BF)
    make_identity(nc, ident_bf[:])
    ident_f = const_pool.tile([128, 128], F32)
    make_identity(nc, ident_f[:])
    ones_col128 = const_pool.tile([128, 1], BF)
    nc.vector.memset(ones_col128[:], 1.0)
    ones_row = const_pool.tile([1, L], BF)
    nc.vector.memset(ones_row[:], 1.0)
    ones_col64 = ones_col128[:L, :]

    # ---- u0[l, b] = sum_i x ----
    u0_ps = psQ.tile([L, B], F32, tag="qr")
    for b in range(B):
        for c in range(C):
            nc.tensor.matmul(u0_ps[:, b : b + 1], xb_sb[:, b, c, :L],
                             ones_col128[:], start=(c == 0), stop=(c == C - 1))
    u0_sb = sbE.tile([L, B], BF, tag="u0")
    nc.vector.tensor_copy(u0_sb[:], u0_ps[:])

    # ---- G_o = W_o^T W_o, chunk-pipelined behind the w DMA; the PSUM->SBUF
    # copies round-robin over DVE/ACT/Pool so copy throughput matches the
    # matmul pipeline ----
    GB = 8
    G_tiles = []
    # rotate G psum tiles over pools that are idle during the head so ~6
    # chunks are in flight and the PSUM->SBUF copies stream back-to-back
    g_pools = [(psL, "lg"), (psU, "u"), (psB, "rqb"), (psP, "pT"),
               (psL, "lg"), (psU, "u")]
    for k in range(O // GB):
        gp, gtag = g_pools[k % len(g_pools)]
        g_ps = gp.tile([L, GB, L], F32, tag=gtag)
        for i in range(GB):
            nc.tensor.matmul(g_ps[:, i, :], wj_ap(k * GB + i),
                             wj_ap(k * GB + i))
        G_k = big.tile([L, GB, L], BF, tag=f"G_{k}")
        if k % 2 == 0:
            nc.vector.tensor_copy(G_k[:], g_ps[:])
        else:
            nc.scalar.copy(G_k[:], g_ps[:])
        G_tiles.append(G_k)

    def G_ap(o):
        return G_tiles[o // GB][:, o % GB, :]

    # iteration-1 p-step, interleaved per G chunk (p = G u0); two o-half
    # tiles so iteration 1's q chain can start on half 0 while the G
    # pipeline is still producing half 1
    pT1_h0 = psP.tile([L, O // 2, B], F32, tag="pT")
    pT1_h1 = psP.tile([L, O // 2, B], F32, tag="pT")
    pT1_half = [pT1_h0, pT1_h1]
    for k in range(O // GB):
        for i in range(GB):
            o = k * GB + i
            nc.tensor.matmul(pT1_half[o // 64][:, o % 64, :], G_ap(o),
                             u0_sb[:])

    def _dummy_out():
        nc.sync.dma_start(out_d[0, 0], ident_f[:1, :J])

    if probe == "P2":
        _dummy_out()
        return

    # ---- routing iterations, software-pipelined in two b-pair streams.
    # Pair B's scalar chain (q -> rsqrt -> rq broadcast -> v) executes under
    # pair A's logits/exp/ua tail, and iteration t+1's p-step + scalar chain
    # executes under iteration t's pair-B tail.  uTZ (l+Z, o, 2) bf16 per
    # pair is the carried state. ----
    I32 = mybir.dt.int32
    st = {}  # (t, pair) -> tiles

    def emit_p(t, p):
        pT = psP.tile([L, O, 2], F32, tag="pT")
        u_prev = st[(t - 1, p)]["uTZ"]
        for o in range(O):
            nc.tensor.matmul(pT[:, o, :], G_ap(o), u_prev[:L, o, :])
        st[(t, p)] = {"pT": pT}

    def emit_scalar_q(t, p):
        s = st[(t, p)]
        # qscr = p * u elementwise (PSUM read), then per-b column matmuls.
        # For t == 1, pT lives in two o-half tiles so half 0's q work starts
        # while the G pipeline is still producing half 1.
        q_ps = psQ.tile([O, 2], F32, tag="qr")
        if t == 1:
            u_in = u0_sb[:, 2 * p : 2 * p + 2].unsqueeze(1).broadcast_to(
                [L, O // 2, 2])
            for h in range(2):
                qscr_h = sbE.tile([L, O // 2, 2], BF, tag=f"qscr{p}h{h}")
                nc.vector.tensor_tensor(out=qscr_h[:], in0=s["pT_halves"][h],
                                        in1=u_in, op=ALU.mult)
                for i in range(2):
                    nc.tensor.matmul(
                        q_ps[h * 64 : (h + 1) * 64, i : i + 1],
                        qscr_h[:, :, i], ones_col64)
        else:
            pT = s["pT"]
            qscr = sbE.tile([L, O, 2], BF, tag=f"qscr{p}")
            nc.vector.tensor_tensor(out=qscr[:], in0=pT[:],
                                    in1=st[(t - 1, p)]["uTZ"][:L],
                                    op=ALU.mult)
            for i in range(2):
                nc.tensor.matmul(q_ps[:, i : i + 1], qscr[:, :, i],
                                 ones_col64)
        s["q_ps"] = q_ps

    def emit_scalar_rest(t, p):
        s = st[(t, p)]
        q_ps = s["q_ps"]
        # rq = rsqrt(q): bit-hack + 1 Newton step on DVE (ACT stays on the
        # single Exp/Copy table)
        s_i = sbE.tile([O, 2], I32, tag=f"rs_s{p}")
        nc.vector.tensor_scalar(out=s_i[:], in0=q_ps[:].bitcast(I32),
                                scalar1=1, scalar2=None,
                                op0=ALU.arith_shift_right)
        y0_i = sbE.tile([O, 2], I32, tag=f"rs_y0{p}")
        nc.vector.tensor_scalar(out=y0_i[:], in0=s_i[:], scalar1=0x5F3759DF,
                                scalar2=-1, op0=ALU.subtract, op1=ALU.mult)
        y0f = y0_i[:].bitcast(F32)
        y2 = sbE.tile([O, 2], F32, tag=f"rs_y2{p}")
        nc.vector.tensor_tensor(out=y2[:], in0=y0f, in1=y0f, op=ALU.mult)
        t1 = sbE.tile([O, 2], F32, tag=f"rs_t1{p}")
        nc.vector.tensor_tensor(out=t1[:], in0=y2[:], in1=q_ps[:],
                                op=ALU.mult)
        t2 = sbE.tile([O, 2], F32, tag=f"rs_t2{p}")
        nc.vector.tensor_scalar(out=t2[:], in0=t1[:], scalar1=-0.5,
                                scalar2=1.5, op0=ALU.mult, op1=ALU.add)
        rq = sbE.tile([O, 2], BF, tag=f"rq{p}")
        nc.vector.tensor_tensor(out=rq[:], in0=y0f, in1=t2[:], op=ALU.mult)
        # rq broadcast over l: transpose each column to a row, ones-col matmul
        rqT_ps = psQ.tile([1, 2, O], BF, tag="qr")
        for i in range(2):
            nc.tensor.transpose(rqT_ps[:, i, :], rq[:, i : i + 1],
                                ident_bf[:])
        rqT_sb = sbE.tile([1, 2, O], BF, tag=f"rqTs{p}")
        nc.vector.tensor_copy(rqT_sb[:], rqT_ps[:])
        rqb = psB.tile([L, 2, O], F32, tag="rqb")
        nc.tensor.matmul(rqb[:], ones_row[:], rqT_sb[:])
        # pT -> SBUF on ACT (feeds the v product; ACT is free pre-exp)
        pT_sb = sb.tile([L, O, 2], BF, tag=f"pT_sb{p}")
        if t == 1:
            for h in range(2):
                nc.scalar.copy(pT_sb[:, h * 64 : (h + 1) * 64, :],
                               s["pT_halves"][h])
        else:
            nc.scalar.copy(pT_sb[:], s["pT"][:])
        s.update(rqb=rqb, pT_sb=pT_sb)

    def emit_v(t, p):
        # per-b v tiles: lg(b0) starts as soon as its own slice is ready
        s = st[(t, p)]
        vs = []
        for i in range(2):
            v_b = sb.tile([L, O], BF, tag=f"v{p}i{i}")
            nc.vector.tensor_tensor(out=v_b[:],
                                    in0=s["pT_sb"][:, :, i],
                                    in1=s["rqb"][:, i, :],
                                    op=ALU.mult)
            vs.append(v_b)
        s["v"] = vs

    def emit_lg_exp(t, p):
        s = st[(t, p)]
        exps = []
        for i in range(2):
            b = 2 * p + i
            lg_ps = psL.tile([128, C, O], F32, tag="lg")
            for c in range(C):
                nc.tensor.matmul(lg_ps[:, c, :], xT_sb[:, b, c, :],
                                 s["v"][i][:])
            exp_sb = sbE.tile([128, C, O], BF, tag=f"exp{b}")
            nc.scalar.activation(exp_sb[:], lg_ps[:], AF.Exp)
            exps.append(exp_sb)
        s["exps"] = exps

    def emit_ua(t, p):
        # per-b u tiles + copies: b0's uTZ slice lands during exp(b1), and
        # only the smaller b1 copy stays on the critical path to p(t+1)
        s = st[(t, p)]
        uTZ = sb.tile([L + 1, O, 2], BF, tag=f"uT{p}")
        for i in range(2):
            b = 2 * p + i
            u_ps = psU.tile([L + 1, O], F32, tag="u")
            for c in range(C):
                nc.tensor.matmul(u_ps[:], xb_sb[:, b, c, :],
                                 s["exps"][i][:, c, :],
                                 start=(c == 0), stop=(c == C - 1))
            nc.vector.tensor_copy(uTZ[:, :, i : i + 1],
                                  u_ps[:].unsqueeze(2))
        s["uTZ"] = uTZ

    def emit_cp(t, p):
        pass

    out_all = sb.tile([O, B, J], F32, tag="out_sb")
    out_view = out_d.transpose([1, 0, 2])

    def emit_final(p):
        s = st[(NITER, p)]
        uTZ = s["uTZ"]
        z_ps = psQ.tile([O, 2, 2], BF, tag="qr")
        for i in range(2):
            nc.tensor.transpose(z_ps[:, i, 0:1], uTZ[L : L + 1, :, i],
                                ident_bf[L : L + 1, L : L + 1])
        rz = sbE.tile([O, 2], F32, tag=f"rz{p}")
        nc.vector.reciprocal(rz[:], z_ps[:, :, 0])
        oT_ps = psP.tile([J, O, 2], F32, tag="pT")
        for o in range(O):
            nc.tensor.matmul(oT_ps[:, o, :], wt_sb[:, o, :],
                             uTZ[:L, o, :])
        oT_sb = sb.tile([J, O, 2], F32, tag=f"oT_sb{p}")
        if p == 0:
            # ACT is idle post-exp and final-A has slack; keep DVE free
            # for pair-B's closing chain
            nc.scalar.copy(oT_sb[:], oT_ps[:])
        else:
            nc.vector.tensor_copy(oT_sb[:], oT_ps[:])
        o_ps = psL.tile([O, 2, J], F32, tag="lg")
        for i in range(2):
            nc.tensor.transpose(o_ps[:, i, :], oT_sb[:, :, i],
                                ident_f[:J, :J])
        rz_bc = rz[:].unsqueeze(2).broadcast_to([O, 2, J])
        nc.vector.tensor_tensor(out=out_all[:, 2 * p : 2 * p + 2, :],
                                in0=o_ps[:], in1=rz_bc, op=ALU.mult)
        nc.sync.dma_start(out_view[:, 2 * p : 2 * p + 2, :],
                          out_all[:, 2 * p : 2 * p + 2, :])

    # iteration-1 p for both pairs comes from the pT1 half tiles
    st[(1, 0)] = {"pT_halves": [h[:, :, 0:2] for h in pT1_half]}
    st[(1, 1)] = {"pT_halves": [h[:, :, 2:4] for h in pT1_half]}

    for t in range(1, NITER + 1):
        emit_scalar_q(t, 0)
        emit_scalar_rest(t, 0)
        if t >= 2:
            # emitted after the A scalar chain so the DVE reorder window
            # cannot interleave the copy into the rsqrt dependency gaps,
            # and pair-B's p burst cannot delay trA on PE
            emit_cp(t - 1, 1)
            emit_p(t, 1)
        emit_v(t, 0)
        emit_scalar_q(t, 1)         # pair-B q before pair-A's logits on PE
        emit_lg_exp(t, 0)
        emit_scalar_rest(t, 1)
        emit_ua(t, 0)
        emit_cp(t, 0)
        emit_v(t, 1)
        emit_lg_exp(t, 1)
        if t < NITER:
            emit_p(t + 1, 0)
            emit_ua(t, 1)
        else:
            # pair-B's tail first: its copies feed the closing final-B
            # chain, while final-A has ~2.5us of slack before its DMA
            emit_ua(t, 1)
            emit_final(0)
        if probe == f"I{t}":
            emit_cp(t, 1)
            _dummy_out()
            return

    emit_cp(NITER, 1)
    emit_final(1)


def build(probe=None):
    nc = bacc.Bacc("TRN2", target_bir_lowering=False, debug=False,
                   enable_asserts=True, num_devices=N_CORES)
    xb_d = nc.dram_tensor("xb", [128, B, C, L + 1], BF, kind="ExternalInput").ap()
    xT_d = nc.dram_tensor("xT", [L, B, C, 128], BF, kind="ExternalInput").ap()
    wj_d = nc.dram_tensor("wj", [J, O, L], BF, kind="ExternalInput").ap()
    wt_d = nc.dram_tensor("wt", [L, O, J], BF, kind="ExternalInput").ap()
    out_d = nc.dram_tensor("out", [B, O, J], F32, kind="ExternalOutput").ap()
    with tile.TileContext(nc) as tc:
        with ExitStack() as ctx:
            _body(ctx, tc, xb_d, xT_d, wj_d, wt_d, out_d, probe=probe)
    nc.compile()
    return nc


_NC = None
LAST_RESULTS = None


def _get_nc():
    global _NC
    if _NC is None:
        _NC = build()
    return _NC


def kernel(x: np.ndarray, weight: np.ndarray) -> np.ndarray:
    assert x.shape == (B_GLOBAL, I, L) and weight.shape == (O, J, L)
    nc = _get_nc()
    bf16 = ml_dtypes.bfloat16
    x = np.ascontiguousarray(x, dtype=np.float32)
    w = np.ascontiguousarray(weight, dtype=np.float32)
    wj = np.ascontiguousarray(w.transpose(1, 0, 2).astype(bf16))   # (j, o, l)
    wt = np.ascontiguousarray(w.transpose(2, 0, 1).astype(bf16))   # (l, o, j)
    in_maps = []
    for i in range(N_CORES):
        xs = x[i * B : (i + 1) * B]                  # (B, I, L)
        xr = xs.reshape(B, 128, C, L)                # i = 4p + c
        xb = np.empty((128, B, C, L + 1), dtype=bf16)
        xb[..., :L] = xr.transpose(1, 0, 2, 3).astype(bf16)
        xb[..., L] = 1.0
        xT = np.ascontiguousarray(xr.transpose(3, 0, 2, 1).astype(bf16))
        in_maps.append({"xb": xb, "xT": xT, "wj": wj, "wt": wt})
    global LAST_RESULTS
    LAST_RESULTS = bass_utils.run_bass_kernel_spmd(
        nc, in_maps, core_ids=list(range(N_CORES)))
    out = np.concatenate(
        [LAST_RESULTS.results[i]["out"] for i in range(N_CORES)], axis=0)
    return out.astype(np.float32)


# revision 56
# speedup vs baseline: 1.0015x; 1.0015x over previous
"""CapsuleLinear (k-means routing) Trainium2 kernel.

Math: priors[b,o,i,j] = sum_l w[o,j,l] x[b,i,l]; 3 rounds of k-means routing
over in_capsules, squash=False.

priors is never materialized.  With G_o = W_o^T W_o (64x64 per out-capsule,
computed on-device once):

    u_0[b,l]   = sum_i x[b,i,l]                  (scale of u is irrelevant)
    per iter:  p = G_o u;  q = u.p = ||W u||^2
               rq = rsqrt(q) via DVE bit-hack + Newton (ACT keeps one table)
               v = p * rq           (v = W^T out_normalized)
               logits[i,o] = sum_l x[b,i,l] v[o,l]
               e = exp(logits)      (softmax Z cancels in v)
               u[o,l] = sum_i e[i,o] x[b,i,l];  Z[o] = sum_i e[i,o]
    output:    out[b,o,:] = W_o u_3[o,:] / Z_3[o]

Sharding: data-parallel over batch, 4 samples/core x 8 cores, weight
replicated, no collectives.  Host passes pre-transposed bf16 layouts
(xb with a ones column, xT, w as (j,o,l) and (l,o,j)) so the device does
no dtype conversion or weight transposition.
"""

import sys

if "/opt/trn_rl_repo" not in sys.path:
    sys.path.insert(0, "/opt/trn_rl_repo")

from contextlib import ExitStack

import ml_dtypes
import numpy as np

import concourse.bacc as bacc
import concourse.bass as bass
import concourse.bass_utils as bass_utils
import concourse.mybir as mybir
import concourse.tile as tile
from concourse.masks import make_identity

BF = mybir.dt.bfloat16
F32 = mybir.dt.float32
AF = mybir.ActivationFunctionType
ALU = mybir.AluOpType

B_GLOBAL = 32
N_CORES = 8
B = B_GLOBAL // N_CORES  # 4 samples per core
O = 128   # out_capsules
I = 512   # in_capsules
J = 64    # out_length
L = 64    # in_length
C = 4     # i-chunks of 128
NITER = 3
WCH = 8   # w DMA chunks
OCH = O // WCH


def _body(ctx: ExitStack, tc: "tile.TileContext", xb_d, xT_d, wj_d, wt_d,
          out_d, probe=None):
    nc = tc.nc

    const_pool = ctx.enter_context(tc.tile_pool(name="const", bufs=1))
    big = ctx.enter_context(tc.tile_pool(name="big", bufs=1))
    sb = ctx.enter_context(tc.tile_pool(name="sb", bufs=2))
    sbE = ctx.enter_context(tc.tile_pool(name="sbE", bufs=3))
    psP = ctx.enter_context(tc.tile_pool(name="psP", bufs=2, space="PSUM"))
    psL = ctx.enter_context(tc.tile_pool(name="psL", bufs=2, space="PSUM"))
    psU = ctx.enter_context(tc.tile_pool(name="psU", bufs=2, space="PSUM"))
    psQ = ctx.enter_context(tc.tile_pool(name="psQ", bufs=1, space="PSUM"))
    psB = ctx.enter_context(tc.tile_pool(name="psB", bufs=1, space="PSUM"))

    # ---- input DMAs (w first: the G pipeline is the head's long pole;
    # x lands while G is still streaming, wt is final-phase only)
    wj_tiles = []
    xb_sb = big.tile([128, B, C, L + 1], BF)
    for k in range(WCH):
        wj_k = big.tile([J, OCH, L], BF, tag=f"wj_{k}")
        nc.sync.dma_start(wj_k[:], wj_d[:, bass.ts(k, OCH), :])
        wj_tiles.append(wj_k)
        if k == WCH // 2 - 1:
            # xb mid-stream so u0 (hence iteration-1's half-0 q chain)
            # unblocks while the second-half w chunks still stream
            nc.sync.dma_start(xb_sb[:], xb_d)
    xT_sb = big.tile([L, B, C, 128], BF)
    nc.sync.dma_start(xT_sb[:], xT_d)
    wt_sb = big.tile([L, O, J], BF)
    nc.sync.dma_start(wt_sb[:], wt_d)

    def wj_ap(o):
        return wj_tiles[o // OCH][:, o % OCH, :]

    # ---- constants ----
    ident_bf = const_pool.tile([128, 128], BF)
    make_identity(nc, ident_bf[:])
    ident_f = const_pool.tile([128, 128], F32)
    make_identity(nc, ident_f[:])
    ones_col128 = const_pool.tile([128, 1], BF)
    nc.vector.memset(ones_col128[:], 1.0)
    ones_row = const_pool.tile([1, L], BF)
    nc.vector.memset(ones_row[:], 1.0)
    ones_col64 = ones_col128[:L, :]

    # ---- u0[l, b] = sum_i x ----
    u0_ps = psQ.tile([L, B], F32, tag="qr")
    for b in range(B):
        for c in range(C):
            nc.tensor.matmul(u0_ps[:, b : b + 1], xb_sb[:, b, c, :L],
                             ones_col128[:], start=(c == 0), stop=(c == C - 1))
    u0_sb = sbE.tile([L, B], BF, tag="u0")
    nc.vector.tensor_copy(u0_sb[:], u0_ps[:])

    # ---- G_o = W_o^T W_o, chunk-pipelined behind the w DMA; the PSUM->SBUF
    # copies round-robin over DVE/ACT/Pool so copy throughput matches the
    # matmul pipeline ----
    GB = 8
    G_tiles = []
    # rotate G psum tiles over pools that are idle during the head so ~6
    # chunks are in flight and the PSUM->SBUF copies stream back-to-back
    g_pools = [(psL, "lg"), (psU, "u"), (psB, "rqb"), (psP, "pT"),
               (psL, "lg"), (psU, "u")]
    for k in range(O // GB):
        gp, gtag = g_pools[k % len(g_pools)]
        g_ps = gp.tile([L, GB, L], F32, tag=gtag)
        for i in range(GB):
            nc.tensor.matmul(g_ps[:, i, :], wj_ap(k * GB + i),
                             wj_ap(k * GB + i))
        G_k = big.tile([L, GB, L], BF, tag=f"G_{k}")
        if k % 2 == 0:
            nc.vector.tensor_copy(G_k[:], g_ps[:])
        else:
            nc.scalar.copy(G_k[:], g_ps[:])
        G_tiles.append(G_k)

    def G_ap(o):
        return G_tiles[o // GB][:, o % GB, :]

    # iteration-1 p-step, interleaved per G chunk (p = G u0); two o-half
    # tiles so iteration 1's q chain can start on half 0 while the G
    # pipeline is still producing half 1
    pT1_h0 = psP.tile([L, O // 2, B], F32, tag="pT")
    pT1_h1 = psP.tile([L, O // 2, B], F32, tag="pT")
    pT1_half = [pT1_h0, pT1_h1]
    for k in range(O // GB):
        for i in range(GB):
            o = k * GB + i
            nc.tensor.matmul(pT1_half[o // 64][:, o % 64, :], G_ap(o),
                             u0_sb[:])

    def _dummy_out():
        nc.sync.dma_start(out_d[0, 0], ident_f[:1, :J])

    if probe == "P2":
        _dummy_out()
        return

    # ---- routing iterations, software-pipelined in two b-pair streams.
    # Pair B's scalar chain (q -> rsqrt -> rq broadcast -> v) executes under
    # pair A's logits/exp/ua tail, and iteration t+1's p-step + scalar chain
    # executes under iteration t's pair-B tail.  uTZ (l+Z, o, 2) bf16 per
    # pair is the carried state. ----
    I32 = mybir.dt.int32
    st = {}  # (t, pair) -> tiles

    def emit_p(t, p):
        pT = psP.tile([L, O, 2], F32, tag="pT")
        u_prev = st[(t - 1, p)]["uTZ"]
        for o in range(O):
            nc.tensor.matmul(pT[:, o, :], G_ap(o), u_prev[:L, o, :])
        st[(t, p)] = {"pT": pT}

    def emit_scalar_q(t, p):
        s = st[(t, p)]
        # qscr = p * u elementwise (PSUM read), then per-b column matmuls.
        # For t == 1, pT lives in two o-half tiles so half 0's q work starts
        # while the G pipeline is still producing half 1.
        q_ps = psQ.tile([O, 2], F32, tag="qr")
        if t == 1:
            u_in = u0_sb[:, 2 * p : 2 * p + 2].unsqueeze(1).broadcast_to(
                [L, O // 2, 2])
            for h in range(2):
                qscr_h = sbE.tile([L, O // 2, 2], BF, tag=f"qscr{p}h{h}")
                nc.vector.tensor_tensor(out=qscr_h[:], in0=s["pT_halves"][h],
                                        in1=u_in, op=ALU.mult)
                for i in range(2):
                    nc.tensor.matmul(
                        q_ps[h * 64 : (h + 1) * 64, i : i + 1],
                        qscr_h[:, :, i], ones_col64)
        else:
            pT = s["pT"]
            qscr = sbE.tile([L, O, 2], BF, tag=f"qscr{p}")
            nc.vector.tensor_tensor(out=qscr[:], in0=pT[:],
                                    in1=st[(t - 1, p)]["uTZ"][:L],
                                    op=ALU.mult)
            for i in range(2):
                nc.tensor.matmul(q_ps[:, i : i + 1], qscr[:, :, i],
                                 ones_col64)
        s["q_ps"] = q_ps

    def emit_scalar_rest(t, p):
        s = st[(t, p)]
        q_ps = s["q_ps"]
        # rq = rsqrt(q): bit-hack + 1 Newton step on DVE (ACT stays on the
        # single Exp/Copy table)
        s_i = sbE.tile([O, 2], I32, tag=f"rs_s{p}")
        nc.vector.tensor_scalar(out=s_i[:], in0=q_ps[:].bitcast(I32),
                                scalar1=1, scalar2=None,
                                op0=ALU.arith_shift_right)
        y0_i = sbE.tile([O, 2], I32, tag=f"rs_y0{p}")
        nc.vector.tensor_scalar(out=y0_i[:], in0=s_i[:], scalar1=0x5F3759DF,
                                scalar2=-1, op0=ALU.subtract, op1=ALU.mult)
        y0f = y0_i[:].bitcast(F32)
        y2 = sbE.tile([O, 2], F32, tag=f"rs_y2{p}")
        nc.vector.tensor_tensor(out=y2[:], in0=y0f, in1=y0f, op=ALU.mult)
        t1 = sbE.tile([O, 2], F32, tag=f"rs_t1{p}")
        nc.vector.tensor_tensor(out=t1[:], in0=y2[:], in1=q_ps[:],
                                op=ALU.mult)
        t2 = sbE.tile([O, 2], F32, tag=f"rs_t2{p}")
        nc.vector.tensor_scalar(out=t2[:], in0=t1[:], scalar1=-0.5,
                                scalar2=1.5, op0=ALU.mult, op1=ALU.add)
        rq = sbE.tile([O, 2], BF, tag=f"rq{p}")
        nc.vector.tensor_tensor(out=rq[:], in0=y0f, in1=t2[:], op=ALU.mult)
        # rq broadcast over l: transpose each column to a row, ones-col matmul
        rqT_ps = psQ.tile([1, 2, O], BF, tag="qr")
        for i in range(2):
            nc.tensor.transpose(rqT_ps[:, i, :], rq[:, i : i + 1],
                                ident_bf[:])
        rqT_sb = sbE.tile([1, 2, O], BF, tag=f"rqTs{p}")
        nc.vector.tensor_copy(rqT_sb[:], rqT_ps[:])
        rqb = psB.tile([L, 2, O], F32, tag="rqb")
        nc.tensor.matmul(rqb[:], ones_row[:], rqT_sb[:])
        # pT -> SBUF on ACT (feeds the v product; ACT is free pre-exp)
        pT_sb = sb.tile([L, O, 2], BF, tag=f"pT_sb{p}")
        if t == 1:
            for h in range(2):
                nc.scalar.copy(pT_sb[:, h * 64 : (h + 1) * 64, :],
                               s["pT_halves"][h])
        else:
            nc.scalar.copy(pT_sb[:], s["pT"][:])
        s.update(rqb=rqb, pT_sb=pT_sb)

    def emit_v(t, p):
        s = st[(t, p)]
        v_sb = sb.tile([L, O, 2], BF, tag=f"v{p}")
        nc.vector.tensor_tensor(out=v_sb[:], in0=s["pT_sb"][:],
                                in1=s["rqb"][:].transpose([0, 2, 1]),
                                op=ALU.mult)
        s["v"] = v_sb

    def emit_lg_exp(t, p):
        s = st[(t, p)]
        exps = []
        for i in range(2):
            b = 2 * p + i
            lg_ps = psL.tile([128, C, O], F32, tag="lg")
            for c in range(C):
                nc.tensor.matmul(lg_ps[:, c, :], xT_sb[:, b, c, :],
                                 s["v"][:, :, i])
            exp_sb = sbE.tile([128, C, O], BF, tag=f"exp{b}")
            nc.scalar.activation(exp_sb[:], lg_ps[:], AF.Exp)
            exps.append(exp_sb)
        s["exps"] = exps

    def emit_ua(t, p):
        # per-b u tiles + copies: b0's uTZ slice lands during exp(b1), and
        # only the smaller b1 copy stays on the critical path to p(t+1)
        s = st[(t, p)]
        uTZ = sb.tile([L + 1, O, 2], BF, tag=f"uT{p}")
        for i in range(2):
            b = 2 * p + i
            u_ps = psU.tile([L + 1, O], F32, tag="u")
            for c in range(C):
                nc.tensor.matmul(u_ps[:], xb_sb[:, b, c, :],
                                 s["exps"][i][:, c, :],
                                 start=(c == 0), stop=(c == C - 1))
            nc.vector.tensor_copy(uTZ[:, :, i : i + 1],
                                  u_ps[:].unsqueeze(2))
        s["uTZ"] = uTZ

    def emit_cp(t, p):
        pass

    out_all = sb.tile([O, B, J], F32, tag="out_sb")
    out_view = out_d.transpose([1, 0, 2])

    def emit_final(p):
        s = st[(NITER, p)]
        uTZ = s["uTZ"]
        z_ps = psQ.tile([O, 2, 2], BF, tag="qr")
        for i in range(2):
            nc.tensor.transpose(z_ps[:, i, 0:1], uTZ[L : L + 1, :, i],
                                ident_bf[L : L + 1, L : L + 1])
        rz = sbE.tile([O, 2], F32, tag=f"rz{p}")
        nc.vector.reciprocal(rz[:], z_ps[:, :, 0])
        oT_ps = psP.tile([J, O, 2], F32, tag="pT")
        for o in range(O):
            nc.tensor.matmul(oT_ps[:, o, :], wt_sb[:, o, :],
                             uTZ[:L, o, :])
        oT_sb = sb.tile([J, O, 2], F32, tag=f"oT_sb{p}")
        if p == 0:
            # ACT is idle post-exp and final-A has slack; keep DVE free
            # for pair-B's closing chain
            nc.scalar.copy(oT_sb[:], oT_ps[:])
        else:
            nc.vector.tensor_copy(oT_sb[:], oT_ps[:])
        o_ps = psL.tile([O, 2, J], F32, tag="lg")
        for i in range(2):
            nc.tensor.transpose(o_ps[:, i, :], oT_sb[:, :, i],
                                ident_f[:J, :J])
        rz_bc = rz[:].unsqueeze(2).broadcast_to([O, 2, J])
        nc.vector.tensor_tensor(out=out_all[:, 2 * p : 2 * p + 2, :],
                                in0=o_ps[:], in1=rz_bc, op=ALU.mult)
        nc.sync.dma_start(out_view[:, 2 * p : 2 * p + 2, :],
                          out_all[:, 2 * p : 2 * p + 2, :])

    # iteration-1 p for both pairs comes from the pT1 half tiles
    st[(1, 0)] = {"pT_halves": [h[:, :, 0:2] for h in pT1_half]}
    st[(1, 1)] = {"pT_halves": [h[:, :, 2:4] for h in pT1_half]}

    for t in range(1, NITER + 1):
        emit_scalar_q(t, 0)
        emit_scalar_rest(t, 0)
        if t >= 2:
            # emitted after the A scalar chain so the DVE reorder window
            # cannot interleave the copy into the rsqrt dependency gaps,
            # and pair-B's p burst cannot delay trA on PE
            emit_cp(t - 1, 1)
            emit_p(t, 1)
        emit_v(t, 0)
        emit_scalar_q(t, 1)         # pair-B q before pair-A's logits on PE
        emit_lg_exp(t, 0)
        emit_scalar_rest(t, 1)
        emit_ua(t, 0)
        emit_cp(t, 0)
        emit_v(t, 1)
        emit_lg_exp(t, 1)
        if t < NITER:
            emit_p(t + 1, 0)
            emit_ua(t, 1)
        else:
            # pair-B's tail first: its copies feed the closing final-B
            # chain, while final-A has ~2.5us of slack before its DMA
            emit_ua(t, 1)
            emit_final(0)
        if probe == f"I{t}":
            emit_cp(t, 1)
            _dummy_out()
            return

    emit_cp(NITER, 1)
    emit_final(1)


def build(probe=None):
    nc = bacc.Bacc("TRN2", target_bir_lowering=False, debug=False,
                   enable_asserts=True, num_devices=N_CORES)
    xb_d = nc.dram_tensor("xb", [128, B, C, L + 1], BF, kind="ExternalInput").ap()
    xT_d = nc.dram_tensor("xT", [L, B, C, 128], BF, kind="ExternalInput").ap()
    wj_d = nc.dram_tensor("wj", [J, O, L], BF, kind="ExternalInput").ap()
    wt_d = nc.dram_tensor("wt", [L, O, J], BF, kind="ExternalInput").ap()
    out_d = nc.dram_tensor("out", [B, O, J], F32, kind="ExternalOutput").ap()
    with tile.TileContext(nc) as tc:
        with ExitStack() as ctx:
            _body(ctx, tc, xb_d, xT_d, wj_d, wt_d, out_d, probe=probe)
    nc.compile()
    return nc


_NC = None
LAST_RESULTS = None


def _get_nc():
    global _NC
    if _NC is None:
        _NC = build()
    return _NC


def kernel(x: np.ndarray, weight: np.ndarray) -> np.ndarray:
    assert x.shape == (B_GLOBAL, I, L) and weight.shape == (O, J, L)
    nc = _get_nc()
    bf16 = ml_dtypes.bfloat16
    x = np.ascontiguousarray(x, dtype=np.float32)
    w = np.ascontiguousarray(weight, dtype=np.float32)
    wj = np.ascontiguousarray(w.transpose(1, 0, 2).astype(bf16))   # (j, o, l)
    wt = np.ascontiguousarray(w.transpose(2, 0, 1).astype(bf16))   # (l, o, j)
    in_maps = []
    for i in range(N_CORES):
        xs = x[i * B : (i + 1) * B]                  # (B, I, L)
        xr = xs.reshape(B, 128, C, L)                # i = 4p + c
        xb = np.empty((128, B, C, L + 1), dtype=bf16)
        xb[..., :L] = xr.transpose(1, 0, 2, 3).astype(bf16)
        xb[..., L] = 1.0
        xT = np.ascontiguousarray(xr.transpose(3, 0, 2, 1).astype(bf16))
        in_maps.append({"xb": xb, "xT": xT, "wj": wj, "wt": wt})
    global LAST_RESULTS
    LAST_RESULTS = bass_utils.run_bass_kernel_spmd(
        nc, in_maps, core_ids=list(range(N_CORES)))
    out = np.concatenate(
        [LAST_RESULTS.results[i]["out"] for i in range(N_CORES)], axis=0)
    return out.astype(np.float32)
